# revision 30
# baseline (speedup 1.0000x reference)
"""Trainium2 Bass kernel for nn_AgeConditionedGraphPriorLoss.

Strategy
--------
logits (2, 32, 96, 96, 96) fp32 is the only large tensor; shard over
(batch B=2) x (four Y-slabs of 24) across 8 NeuronCores; each core keeps
full X so the flip/swap symmetry term is shard-local.  Host pre-transposes
each shard to [X, 128, C, VT] fp8-e4m3 (voxel v = y*Z+z -> (vt, part));
fp8 halves the HBM stream and costs ~nothing (softmax self-normalizes the
quantization bias; measured end-to-end rel err 9e-6).

Per-core pipeline, one (ascending, descending) x-chunk pair per iteration
so flip partners align element-wise:
  ACT   exp only, ONE fat instr/iter writing channel-major e
        (transposed APs are free on ACT)                       ~48 us busy
  DVE   channel-sum tree (5 contiguous bf16 TT halvings at 2x),
        reciprocal_approx_fast (custom op, ~5x faster than the
        iterative-divide reciprocal), t cast, one fat normalize
        mul (2x broadcast TT), TT-max for the symmetry term     ~87 us busy
  PE    gram only: one M=32 matmul per 128-voxel tile,
        round-robined over the four 32-col PE array groups via
        tile_position, all accumulating in one PSUM group       ~65 us busy
  GPS   idle on purpose: its stock TT is ~4x slower than DVE AND it
        contends with DVE for the shared SBUF port (measured 2x DVE
        slowdowns when active)

Symmetry term via the softmax identity (no sub, no abs, no 1x
accumulate ops):  sum|p_a - p_b'| = 2*sum max(p_a, p_b') - n_voxels.

CHUNKS tapers the first/last iterations (1-2 x-slabs instead of 4) to
prime/drain the cross-iteration pipeline faster.  The per-iteration
max(p_a, p_b') sums all run on the ACT (Copy activation + accum_out),
each deferred SYM_DEFER iterations so it fills the ACT's per-iteration
slack behind later exps (strict in-order FIFO) instead of delaying
them; the PE queue stays pure gram work.  The tiny O(C^2) final loss
math runs on host in numpy.

Baseline 119.9 us (DVE 97.5 busy, ACT 86.7 busy); this version measures
~108 us in the good mode (DVE ~89 busy = the TT-2x floor, ACT ~64,
PE ~65).  The box is HW-throttled (util limit ~0.5) and bimodal:
identical NEFFs read ~107-110 us or ~128-131 us run to run.
"""

import os
import sys

import numpy as np
from contextlib import ExitStack

# kernel.py is graded from a bare directory: make the concourse/bass stack
# importable regardless of cwd
for _p in ("/opt/trn_rl_repo", "/root/.axon_site/_ro/trn_rl_repo"):
    if os.path.isdir(_p) and _p not in sys.path:
        sys.path.append(_p)

# ---- problem constants (hardcoded per harness contract) ----
B = 2
C = 32
X = 96
Y = 96
Z = 96
N_CORES = 8
YQ = Y // 4          # y-slab per core
P = 128              # SBUF partitions

LAMBDA_VOLUME = 0.2
LAMBDA_WEIGHTED_ADJ = 0.15
LAMBDA_SYM = 0.05
AGE_MAX = 100.0
EPS_ROW = 1e-8
EPS_STD = 1e-6

CM = 4               # max x-slabs per direction per iteration
# small first/last iterations prime and drain the cross-iteration pipeline
# quickly (the measured stalls were ~15 us of ramp-up waiting on the first
# big DMA+exp pair and ~14 us of ramp-down on the last serial tree tail)
CHUNKS = [1, 1, 2, 2, 2] + [4] * 9 + [2, 1, 1]
SYM_ACT_TAIL = 6     # last N iterations sum m on the idle ACT instead of the PE
SYM_MM_COLS = 288    # moving-column chunk for the sym-sum ones-matmuls


def build_nc(Cc=C, XS=X, YQc=YQ, Zc=Z):
    """Build the per-core Bass program (SPMD: same program on all cores).

    Input : "lg"      [XS, 128, Cc, VT] bf16   (this core's logits shard)
    Output: "a_out"   [128, Cc] fp32           (packed gram-matrix blocks)
            "sym_out" [128, NITER] fp32        (partial abs-diff sums)
    """
    import concourse.bass as bass
    import concourse.bacc as bacc
    import concourse.tile as tile
    from concourse import mybir

    f32 = mybir.dt.float32
    bf16 = mybir.dt.bfloat16

    NV = YQc * Zc                 # voxels per x-slab
    assert NV % P == 0
    VT = NV // P                  # 128-voxel tiles per x-slab (18)
    assert sum(CHUNKS) * 2 == XS
    assert all(ch <= CM and (ch * Cc * VT) % SYM_MM_COLS == 0 for ch in CHUNKS)
    CH = Cc // 2
    JM = 2 * CM                   # max merged (direction, x) dim per iter
    CV = Cc * VT                  # 576 elems per (x, partition)

    nc = bacc.Bacc("TRN2", target_bir_lowering=False)
    f8 = mybir.dt.float8e4
    lg = nc.dram_tensor("lg", [XS, P, Cc, VT], f8, kind="ExternalInput")
    a_out = nc.dram_tensor("a_out", [P, 4 * Cc], f32, kind="ExternalOutput")
    symc_out = nc.dram_tensor("symc_out", [P, SYM_ACT_TAIL], f32, kind="ExternalOutput")
    sym_out = nc.dram_tensor("sym_out", [1, SYM_MM_COLS], f32, kind="ExternalOutput")

    SLAB = P * CV                 # elements per x-slab
    lg_dma_ring = []

    def load_chunk(t, jx0, x0, descending, count):
        # fill t[:, jx0:jx0+count] with count x-slabs starting at x0
        # (ascending or descending) so partner slabs line up element-wise.
        sx = -SLAB if descending else SLAB
        src = bass.AP(
            tensor=lg,
            offset=x0 * SLAB,
            ap=[[CV, P], [sx, count], [1, CV]],
        )
        d = nc.sync.dma_start(out=t[:, jx0 : jx0 + count, :, :], in_=src)
        lg_dma_ring.append(d)

    with tile.TileContext(nc) as tc, ExitStack() as ctx:
        lg_pool = ctx.enter_context(tc.tile_pool(name="lg", bufs=4))
        e_pool = ctx.enter_context(tc.tile_pool(name="e", bufs=4))
        p_pool = ctx.enter_context(tc.tile_pool(name="p", bufs=6))
        st_pool = ctx.enter_context(tc.tile_pool(name="st", bufs=3))
        sm_pool = ctx.enter_context(tc.tile_pool(name="sm", bufs=3))
        d_pool = ctx.enter_context(tc.tile_pool(name="d", bufs=7))
        one_pool = ctx.enter_context(tc.tile_pool(name="one", bufs=1))
        ps_pool = ctx.enter_context(tc.tile_pool(name="ps", bufs=1, space="PSUM"))

        a_psum = ps_pool.tile([P, 4 * Cc], f32)
        sym_psum = ps_pool.tile([1, SYM_MM_COLS], f32)
        a_sb = one_pool.tile([P, 4 * Cc], f32)
        sym_sb = one_pool.tile([1, SYM_MM_COLS], f32)
        sym_cols = one_pool.tile([P, SYM_ACT_TAIL], f32)
        zeros_ws = one_pool.tile([P, SYM_MM_COLS], bf16)
        nc.vector.memset(zeros_ws[:], 0.0)
        ones_col = one_pool.tile([P, 1], bf16)
        nc.vector.memset(ones_col[:], 1.0)

        # open the PSUM accumulation groups with full-width zero matmuls so
        # every accumulated element is started/zeroed exactly once; the
        # per-tile matmuls below all use start=False.
        nc.tensor.matmul(
            a_psum[:, 0 : 4 * Cc], zeros_ws[:, 0:P], zeros_ws[:, 0 : 4 * Cc],
            start=True, stop=False,
        )
        nc.tensor.matmul(
            sym_psum[0:1, 0:SYM_MM_COLS], zeros_ws[:, 0:1],
            zeros_ws[:, 0:SYM_MM_COLS],
            start=True, stop=False,
        )

        mm_count = 0
        asc_used = 0
        desc_used = 0
        deferred_sym = []
        for it, ch in enumerate(CHUNKS):
            J = 2 * ch
            xa = asc_used
            asc_used += ch
            xb_hi = XS - 1 - desc_used    # descending start for partner chunk
            desc_used += ch

            # one tile holds both directions: jx 0..ch-1 ascending from xa,
            # jx ch..2*ch-1 descending from xb_hi (flip partners align).
            lg_t = lg_pool.tile([P, JM, Cc, VT], f8, tag="lg")
            load_chunk(lg_t, 0, xa, False, ch)
            load_chunk(lg_t, ch, xb_hi, True, ch)

            # e is channel-major [P, c, jx, v] so every tree level is a
            # contiguous halving over c; exp transposes for free on ACT
            # (1x rate regardless of AP).
            e_t = e_pool.tile([P, Cc, JM, VT], bf16, tag="e")
            nc.scalar.activation(
                out=e_t[:, :, 0:J, :].transpose([0, 2, 1, 3]),
                in_=lg_t[:, 0:J, :, :],
                func=mybir.ActivationFunctionType.Exp,
            )

            # channel-sum tree 32 -> 16 -> 8 -> 4 -> 2 -> 1 on DVE
            # (bf16 TT at 2x; last level to fp32 for the reciprocal).
            st1 = st_pool.tile([P, CH, JM, VT], bf16, tag="st1")
            st2 = st_pool.tile([P, CH // 2, JM, VT], bf16, tag="st2")
            st3 = st_pool.tile([P, CH // 4, JM, VT], bf16, tag="st3")
            st4 = st_pool.tile([P, CH // 8, JM, VT], bf16, tag="st4")
            s_f = sm_pool.tile([P, JM, VT], f32, tag="s")
            nc.vector.tensor_add(
                st1[:, :, 0:J, :], e_t[:, 0:CH, 0:J, :], e_t[:, CH:Cc, 0:J, :]
            )
            c = CH // 2
            nc.vector.tensor_add(
                st2[:, :, 0:J, :], st1[:, 0:c, 0:J, :], st1[:, c : 2 * c, 0:J, :]
            )
            c //= 2
            nc.vector.tensor_add(
                st3[:, :, 0:J, :], st2[:, 0:c, 0:J, :], st2[:, c : 2 * c, 0:J, :]
            )
            c //= 2
            nc.vector.tensor_add(
                st4[:, :, 0:J, :], st3[:, 0:c, 0:J, :], st3[:, c : 2 * c, 0:J, :]
            )
            nc.vector.tensor_add(
                s_f[:, 0:J, :], st4[:, 0, 0:J, :], st4[:, 1, 0:J, :]
            )

            # t = 1/s: approx reciprocal (~51 ULP, ~5x faster than the
            # iterative-divide reciprocal()), then cast to bf16 for the mul.
            t_f = sm_pool.tile([P, JM, VT], f32, tag="tf")
            t_b = sm_pool.tile([P, JM, VT], bf16, tag="tb")
            nc.vector.reciprocal_approx_fast(
                out=t_f[:, 0:J, :], in_=s_f[:, 0:J, :]
            )
            nc.vector.tensor_copy(out=t_b[:, 0:J, :], in_=t_f[:, 0:J, :])

            # normalize into voxel-major p [P, jx, c, v] (the layout the
            # matmuls and the symmetry term want); t broadcast over c
            # (stride-0 middle dim keeps the 2x TT mode).
            p_t = p_pool.tile([P, JM, Cc, VT], bf16, tag="p")
            t_bc = (
                t_b[:, 0:J, :].unsqueeze(2).broadcast_to([P, J, Cc, VT])
            )
            nc.vector.tensor_mul(
                p_t[:, 0:J, :, :],
                e_t[:, :, 0:J, :].transpose([0, 2, 1, 3]),
                t_bc,
            )

            # symmetry term via the softmax-row-sum identity:
            #   sum |p_a - p_b'| = 2 * sum max(p_a, p_b') - n_voxels
            # (since max+min = p_a+p_b' and softmax columns each sum to 1).
            # One TT-max pass replaces sub+abs entirely, max of two bf16
            # values is exact, and the sum runs on the PE as ones-stationary
            # matmuls accumulating into a [1, SYM_MM_COLS] PSUM strip.
            m_t = d_pool.tile([P, CM, Cc, VT], bf16, tag="m")
            nc.vector.tensor_max(
                m_t[:, 0:ch, 0:CH, :],
                p_t[:, 0:ch, 0:CH, :],
                p_t[:, ch:J, CH:Cc, :],
            )
            nc.vector.tensor_max(
                m_t[:, 0:ch, CH:Cc, :],
                p_t[:, 0:ch, CH:Cc, :],
                p_t[:, ch:J, 0:CH, :],
            )
            # gram matmuls: one M=32 matmul per vtile; vtiles round-robin
            # over the four 32-column PE array groups via tile_position so
            # they run concurrently (the ISA allows only one free dim on the
            # stationary AP, so batched multi-vtile stationaries are out).
            # Emitted before the sym matmuls so the PE never waits on the
            # TT-max output.
            for jx in range(J):
                for vt in range(VT):
                    pv = p_t[:, jx, :, vt]
                    g = vt % 4
                    nc.tensor.matmul(
                        a_psum[32 * g : 32 * g + 32, 32 * g : 32 * g + 32],
                        pv,
                        pv,
                        start=False,
                        stop=False,
                        tile_position=(0, 32 * g),
                        skip_group_check=True,
                    )
                    mm_count += 1

            m_flat = m_t[:, 0:ch, :, :].rearrange("p x c v -> p (x c v)")
            tail_i = it - (len(CHUNKS) - SYM_ACT_TAIL)
            if tail_i < 0:
                for k in range((ch * Cc * VT) // SYM_MM_COLS):
                    nc.tensor.matmul(
                        sym_psum[0:1, 0:SYM_MM_COLS],
                        ones_col[:, 0:1],
                        m_flat[:, SYM_MM_COLS * k : SYM_MM_COLS * (k + 1)],
                        start=False,
                        stop=False,
                        tile_position=(0, 0),
                        skip_group_check=True,
                    )
            else:
                # tail iterations: defer the m sums; they are emitted after
                # the loop so they queue on the ACT behind ALL exps (the ACT
                # is a strict in-order FIFO) and run in its idle tail while
                # the PE drains its gram backlog.
                deferred_sym.append((tail_i, m_flat))

        for tail_i, m_flat in deferred_sym:
            ms = d_pool.tile([P, CM * Cc * VT], bf16, tag="ms")
            nc.scalar.activation(
                out=ms[:, 0 : m_flat.shape[1]],
                in_=m_flat,
                func=mybir.ActivationFunctionType.Copy,
                accum_out=sym_cols[:, tail_i : tail_i + 1],
            )
        assert mm_count == XS * VT
        # close the accumulation groups (adds zeros, flips stop)
        nc.tensor.matmul(
            a_psum[:, 0 : 4 * Cc], zeros_ws[:, 0:P], zeros_ws[:, 0 : 4 * Cc],
            start=False, stop=True,
        )
        nc.tensor.matmul(
            sym_psum[0:1, 0:SYM_MM_COLS], zeros_ws[:, 0:1],
            zeros_ws[:, 0:SYM_MM_COLS],
            start=False, stop=True,
        )
        nc.vector.tensor_copy(out=a_sb[:], in_=a_psum[:])
        nc.vector.tensor_copy(out=sym_sb[:], in_=sym_psum[:])
        nc.sync.dma_start(out=a_out[:], in_=a_sb[:])
        nc.sync.dma_start(out=sym_out[:], in_=sym_sb[:])
        nc.sync.dma_start(out=symc_out[:], in_=sym_cols[:])

    # The HWDGE pseudo-DMA has a single sync-wait slot, but a recycled load
    # buffer carries both a WAR wait (previous exp read, Activation sem) and
    # a WAW wait (previous fill, DMAHW sem).  All SP-issued HWDGE DMAs share
    # one physical FIFO ring (qSPDynamicHW), so same-ring WAW ordering is
    # guaranteed by hardware per SDMA engine; drop the redundant DMAHW wait.
    for d in lg_dma_ring:
        si = d.ins.sync_info
        if si is None or si.on_wait is None:
            continue
        ws = list(si.on_wait)
        if len(ws) > 1:
            keep = [w for w in ws if not (w.ant_name or "").startswith("DMAHW")]
            if keep and len(keep) < len(ws):
                si.on_wait = keep

    nc.compile()
    return nc


def _finish_loss(A_b, vol_b, sym_total, age, w_young, w_old,
                 vol_means_young, vol_means_old, vol_stds_young, vol_stds_old,
                 prior_adj):
    """Host-side tiny final math (numpy, float64 internally)."""
    alpha = np.clip(age.astype(np.float64) / AGE_MAX, 0.0, 1.0)  # (B,1)

    eye = np.eye(C)
    A = A_b * (1.0 - eye)[None]                                   # zero diag
    W = (1.0 - alpha)[:, :, None] * w_young[None] + alpha[:, :, None] * w_old[None]
    Aw = (A * W).mean(axis=0)
    Aw = Aw / np.clip(Aw.sum(axis=1, keepdims=True), EPS_ROW, None)
    prior = prior_adj * (1.0 - eye)
    prior = prior / np.clip(prior.sum(axis=1, keepdims=True), EPS_ROW, None)
    loss_adj = np.mean(np.abs(Aw - prior))

    means = (1.0 - alpha) * vol_means_young[None] + alpha * vol_means_old[None]
    stds = (1.0 - alpha) * vol_stds_young[None] + alpha * vol_stds_old[None]
    r = (vol_b - means) / (stds + EPS_STD)
    ar = np.abs(r)
    loss_vol = np.mean(np.where(ar < 1.0, 0.5 * r * r, ar - 0.5))

    loss_sym = sym_total / float(B * C * X * Y * Z)

    total = (LAMBDA_WEIGHTED_ADJ * loss_adj
             + LAMBDA_VOLUME * loss_vol
             + LAMBDA_SYM * loss_sym)
    return np.float32(total)


def _shard_for_core(logits, b, q, Cc=C, XS=X, YQc=YQ, Zc=Z):
    """Slice one core's shard and lay it out as [XS, 128, Cc, VT] bf16 with
    voxel v = y*Zc + z mapped to (vt, part) = (v // 128, v % 128)."""
    NV = YQc * Zc
    VT = NV // P
    sh = logits[b, :, :, q * YQc : (q + 1) * YQc, :]      # [C, XS, YQ, Z]
    sh = sh.reshape(Cc, XS, VT, P)                        # v -> (vt, part)
    sh = sh.transpose(1, 3, 0, 2)                         # [XS, part, C, VT]
    import ml_dtypes
    return np.ascontiguousarray(np.asarray(sh, dtype=np.float32).astype(ml_dtypes.float8_e4m3))


_CACHE = {}


def kernel(logits, age, w_young, w_old, vol_means_young, vol_means_old,
           vol_stds_young, vol_stds_old, prior_adj, perm):
    from concourse.bass_utils import run_bass_kernel_spmd

    logits = np.asarray(logits, dtype=np.float32)

    if "nc" not in _CACHE:
        _CACHE["nc"] = build_nc()
    nc = _CACHE["nc"]

    in_maps = []
    for core in range(N_CORES):
        b = core // 4
        q = core % 4
        in_maps.append({"lg": _shard_for_core(logits, b, q)})

    res = run_bass_kernel_spmd(nc, in_maps, core_ids=list(range(N_CORES)))
    _CACHE["last_results"] = res

    A_b = np.zeros((B, C, C), dtype=np.float64)
    sym_total = 0.0
    for core in range(N_CORES):
        b = core // 4
        a_full = res.results[core]["a_out"].astype(np.float64)  # [128, 4*C]
        for j in range(4):
            A_b[b] += a_full[j * C : (j + 1) * C, j * C : (j + 1) * C]
        n_vox_core = X * YQ * Z  # voxels per core (each contributes sum_c p = 1)
        sum_max = float(res.results[core]["sym_out"].astype(np.float64).sum())
        sum_max += float(res.results[core]["symc_out"].astype(np.float64).sum())
        sym_total += 2.0 * (2.0 * sum_max - float(n_vox_core))
    vol_b = A_b.sum(axis=2)  # softmax rows sum to 1 -> row sums give volumes

    return _finish_loss(
        A_b, vol_b, sym_total,
        np.asarray(age), np.asarray(w_young), np.asarray(w_old),
        np.asarray(vol_means_young), np.asarray(vol_means_old),
        np.asarray(vol_stds_young), np.asarray(vol_stds_old),
        np.asarray(prior_adj),
    )


# revision 31
# speedup vs baseline: 1.0142x; 1.0142x over previous
"""Trainium2 Bass kernel for nn_AgeConditionedGraphPriorLoss.

Strategy
--------
logits (2, 32, 96, 96, 96) fp32 is the only large tensor; shard over
(batch B=2) x (four Y-slabs of 24) across 8 NeuronCores; each core keeps
full X so the flip/swap symmetry term is shard-local.  Host pre-transposes
each shard to [X, 128, C, VT] fp8-e4m3 (voxel v = y*Z+z -> (vt, part));
fp8 halves the HBM stream and costs ~nothing (softmax self-normalizes the
quantization bias; measured end-to-end rel err 9e-6).

Per-core pipeline, one (ascending, descending) x-chunk pair per iteration
so flip partners align element-wise:
  ACT   exp only, ONE fat instr/iter writing channel-major e
        (transposed APs are free on ACT)                       ~48 us busy
  DVE   channel-sum tree (5 contiguous bf16 TT halvings at 2x),
        reciprocal_approx_fast (custom op, ~5x faster than the
        iterative-divide reciprocal), t cast, one fat normalize
        mul (2x broadcast TT), TT-max for the symmetry term     ~87 us busy
  PE    gram: one M=32 matmul per 128-voxel tile, round-robined
        over the four 32-col PE groups via tile_position, all
        accumulating in one PSUM group; sym: ones-stationary
        matmuls summing max(p_a,p_b') into a [1,288] PSUM strip ~67 us busy
  GPS   idle on purpose: its stock TT is ~4x slower than DVE AND it
        contends with DVE for the shared SBUF port (measured 2x DVE
        slowdowns when active)

Symmetry term via the softmax identity (no sub, no abs, no 1x
accumulate ops):  sum|p_a - p_b'| = 2*sum max(p_a, p_b') - n_voxels.

CHUNKS tapers the first/last iterations (1-2 x-slabs instead of 4) to
prime/drain the cross-iteration pipeline faster, and the last
SYM_ACT_TAIL iterations sum m on the ACT instead (Copy + accum_out,
emitted after the loop so they queue behind all exps on the in-order
ACT and run in its idle tail while the PE drains its gram backlog).
The tiny O(C^2) final loss math runs on host in numpy.

Baseline 119.9 us (DVE 97.5 busy, ACT 86.7 busy); this version measured
107.1-108.3 us across good-mode runs (DVE ~89 busy = the TT-2x floor).
The box is HW-throttled (util limit ~0.5) and bimodal: identical NEFFs
read ~107-110 us or ~128-131 us run to run.
"""

import os
import sys

import numpy as np
from contextlib import ExitStack

# kernel.py is graded from a bare directory: make the concourse/bass stack
# importable regardless of cwd
for _p in ("/opt/trn_rl_repo", "/root/.axon_site/_ro/trn_rl_repo"):
    if os.path.isdir(_p) and _p not in sys.path:
        sys.path.append(_p)

# ---- problem constants (hardcoded per harness contract) ----
B = 2
C = 32
X = 96
Y = 96
Z = 96
N_CORES = 8
YQ = Y // 4          # y-slab per core
P = 128              # SBUF partitions

LAMBDA_VOLUME = 0.2
LAMBDA_WEIGHTED_ADJ = 0.15
LAMBDA_SYM = 0.05
AGE_MAX = 100.0
EPS_ROW = 1e-8
EPS_STD = 1e-6

CM = 4               # max x-slabs per direction per iteration
# small first/last iterations prime and drain the cross-iteration pipeline
# quickly (the measured stalls were ~15 us of ramp-up waiting on the first
# big DMA+exp pair and ~14 us of ramp-down on the last serial tree tail)
CHUNKS = [1, 1, 2, 2, 2] + [4] * 9 + [2, 1, 1]
SYM_ACT_TAIL = 6     # last N iterations sum m on the idle ACT instead of the PE
SYM_MM_COLS = 288    # moving-column chunk for the sym-sum ones-matmuls


def build_nc(Cc=C, XS=X, YQc=YQ, Zc=Z):
    """Build the per-core Bass program (SPMD: same program on all cores).

    Input : "lg"      [XS, 128, Cc, VT] bf16   (this core's logits shard)
    Output: "a_out"   [128, Cc] fp32           (packed gram-matrix blocks)
            "sym_out" [128, NITER] fp32        (partial abs-diff sums)
    """
    import concourse.bass as bass
    import concourse.bacc as bacc
    import concourse.tile as tile
    from concourse import mybir

    f32 = mybir.dt.float32
    bf16 = mybir.dt.bfloat16

    NV = YQc * Zc                 # voxels per x-slab
    assert NV % P == 0
    VT = NV // P                  # 128-voxel tiles per x-slab (18)
    assert sum(CHUNKS) * 2 == XS
    assert all(ch <= CM and (ch * Cc * VT) % SYM_MM_COLS == 0 for ch in CHUNKS)
    CH = Cc // 2
    JM = 2 * CM                   # max merged (direction, x) dim per iter
    CV = Cc * VT                  # 576 elems per (x, partition)

    nc = bacc.Bacc("TRN2", target_bir_lowering=False)
    f8 = mybir.dt.float8e4
    lg = nc.dram_tensor("lg", [XS, P, Cc, VT], f8, kind="ExternalInput")
    a_out = nc.dram_tensor("a_out", [P, 4 * Cc], f32, kind="ExternalOutput")
    symc_out = nc.dram_tensor("symc_out", [P, SYM_ACT_TAIL], f32, kind="ExternalOutput")
    sym_out = nc.dram_tensor("sym_out", [1, SYM_MM_COLS], f32, kind="ExternalOutput")

    SLAB = P * CV                 # elements per x-slab
    lg_dma_ring = []

    def load_chunk(t, jx0, x0, descending, count):
        # fill t[:, jx0:jx0+count] with count x-slabs starting at x0
        # (ascending or descending) so partner slabs line up element-wise.
        sx = -SLAB if descending else SLAB
        src = bass.AP(
            tensor=lg,
            offset=x0 * SLAB,
            ap=[[CV, P], [sx, count], [1, CV]],
        )
        d = nc.sync.dma_start(out=t[:, jx0 : jx0 + count, :, :], in_=src)
        lg_dma_ring.append(d)

    with tile.TileContext(nc) as tc, ExitStack() as ctx:
        lg_pool = ctx.enter_context(tc.tile_pool(name="lg", bufs=4))
        e_pool = ctx.enter_context(tc.tile_pool(name="e", bufs=4))
        p_pool = ctx.enter_context(tc.tile_pool(name="p", bufs=6))
        st_pool = ctx.enter_context(tc.tile_pool(name="st", bufs=3))
        sm_pool = ctx.enter_context(tc.tile_pool(name="sm", bufs=3))
        d_pool = ctx.enter_context(tc.tile_pool(name="d", bufs=7))
        one_pool = ctx.enter_context(tc.tile_pool(name="one", bufs=1))
        ps_pool = ctx.enter_context(tc.tile_pool(name="ps", bufs=1, space="PSUM"))

        a_psum = ps_pool.tile([P, 4 * Cc], f32)
        sym_psum = ps_pool.tile([1, SYM_MM_COLS], f32)
        a_sb = one_pool.tile([P, 4 * Cc], f32)
        sym_sb = one_pool.tile([1, SYM_MM_COLS], f32)
        sym_cols = one_pool.tile([P, SYM_ACT_TAIL], f32)
        zeros_ws = one_pool.tile([P, SYM_MM_COLS], bf16)
        nc.vector.memset(zeros_ws[:], 0.0)
        ones_col = one_pool.tile([P, 1], bf16)
        nc.vector.memset(ones_col[:], 1.0)

        # open the PSUM accumulation groups with full-width zero matmuls so
        # every accumulated element is started/zeroed exactly once; the
        # per-tile matmuls below all use start=False.
        nc.tensor.matmul(
            a_psum[:, 0 : 4 * Cc], zeros_ws[:, 0:P], zeros_ws[:, 0 : 4 * Cc],
            start=True, stop=False,
        )
        nc.tensor.matmul(
            sym_psum[0:1, 0:SYM_MM_COLS], zeros_ws[:, 0:1],
            zeros_ws[:, 0:SYM_MM_COLS],
            start=True, stop=False,
        )

        mm_count = 0
        asc_used = 0
        desc_used = 0
        deferred_sym = []
        for it, ch in enumerate(CHUNKS):
            J = 2 * ch
            xa = asc_used
            asc_used += ch
            xb_hi = XS - 1 - desc_used    # descending start for partner chunk
            desc_used += ch

            # one tile holds both directions: jx 0..ch-1 ascending from xa,
            # jx ch..2*ch-1 descending from xb_hi (flip partners align).
            lg_t = lg_pool.tile([P, JM, Cc, VT], f8, tag="lg")
            load_chunk(lg_t, 0, xa, False, ch)
            load_chunk(lg_t, ch, xb_hi, True, ch)

            # e is channel-major [P, c, jx, v] so every tree level is a
            # contiguous halving over c; exp transposes for free on ACT
            # (1x rate regardless of AP).
            e_t = e_pool.tile([P, Cc, JM, VT], bf16, tag="e")
            nc.scalar.activation(
                out=e_t[:, :, 0:J, :].transpose([0, 2, 1, 3]),
                in_=lg_t[:, 0:J, :, :],
                func=mybir.ActivationFunctionType.Exp,
            )

            # channel-sum tree 32 -> 16 -> 8 -> 4 -> 2 -> 1 on DVE
            # (bf16 TT at 2x; last level to fp32 for the reciprocal).
            st1 = st_pool.tile([P, CH, JM, VT], bf16, tag="st1")
            st2 = st_pool.tile([P, CH // 2, JM, VT], bf16, tag="st2")
            st3 = st_pool.tile([P, CH // 4, JM, VT], bf16, tag="st3")
            st4 = st_pool.tile([P, CH // 8, JM, VT], bf16, tag="st4")
            s_f = sm_pool.tile([P, JM, VT], f32, tag="s")
            nc.vector.tensor_add(
                st1[:, :, 0:J, :], e_t[:, 0:CH, 0:J, :], e_t[:, CH:Cc, 0:J, :]
            )
            c = CH // 2
            nc.vector.tensor_add(
                st2[:, :, 0:J, :], st1[:, 0:c, 0:J, :], st1[:, c : 2 * c, 0:J, :]
            )
            c //= 2
            nc.vector.tensor_add(
                st3[:, :, 0:J, :], st2[:, 0:c, 0:J, :], st2[:, c : 2 * c, 0:J, :]
            )
            c //= 2
            nc.vector.tensor_add(
                st4[:, :, 0:J, :], st3[:, 0:c, 0:J, :], st3[:, c : 2 * c, 0:J, :]
            )
            nc.vector.tensor_add(
                s_f[:, 0:J, :], st4[:, 0, 0:J, :], st4[:, 1, 0:J, :]
            )

            # t = 1/s: approx reciprocal (~51 ULP, ~5x faster than the
            # iterative-divide reciprocal()), then cast to bf16 for the mul.
            t_f = sm_pool.tile([P, JM, VT], f32, tag="tf")
            t_b = sm_pool.tile([P, JM, VT], bf16, tag="tb")
            nc.vector.reciprocal_approx_fast(
                out=t_f[:, 0:J, :], in_=s_f[:, 0:J, :]
            )
            nc.vector.tensor_copy(out=t_b[:, 0:J, :], in_=t_f[:, 0:J, :])

            # normalize into voxel-major p [P, jx, c, v] (the layout the
            # matmuls and the symmetry term want); t broadcast over c
            # (stride-0 middle dim keeps the 2x TT mode).
            p_t = p_pool.tile([P, JM, Cc, VT], bf16, tag="p")
            t_bc = (
                t_b[:, 0:J, :].unsqueeze(2).broadcast_to([P, J, Cc, VT])
            )
            nc.vector.tensor_mul(
                p_t[:, 0:J, :, :],
                e_t[:, :, 0:J, :].transpose([0, 2, 1, 3]),
                t_bc,
            )

            # symmetry term via the softmax-row-sum identity:
            #   sum |p_a - p_b'| = 2 * sum max(p_a, p_b') - n_voxels
            # (since max+min = p_a+p_b' and softmax columns each sum to 1).
            # One TT-max pass replaces sub+abs entirely, max of two bf16
            # values is exact, and the sum runs on the PE as ones-stationary
            # matmuls accumulating into a [1, SYM_MM_COLS] PSUM strip.
            m_t = d_pool.tile([P, CM, Cc, VT], bf16, tag="m")
            nc.vector.tensor_max(
                m_t[:, 0:ch, 0:CH, :],
                p_t[:, 0:ch, 0:CH, :],
                p_t[:, ch:J, CH:Cc, :],
            )
            nc.vector.tensor_max(
                m_t[:, 0:ch, CH:Cc, :],
                p_t[:, 0:ch, CH:Cc, :],
                p_t[:, ch:J, 0:CH, :],
            )
            # gram matmuls: one M=32 matmul per vtile; vtiles round-robin
            # over the four 32-column PE array groups via tile_position so
            # they run concurrently (the ISA allows only one free dim on the
            # stationary AP, so batched multi-vtile stationaries are out).
            # Emitted before the sym matmuls so the PE never waits on the
            # TT-max output.
            for jx in range(J):
                for vt in range(VT):
                    pv = p_t[:, jx, :, vt]
                    g = vt % 4
                    nc.tensor.matmul(
                        a_psum[32 * g : 32 * g + 32, 32 * g : 32 * g + 32],
                        pv,
                        pv,
                        start=False,
                        stop=False,
                        tile_position=(0, 32 * g),
                        skip_group_check=True,
                    )
                    mm_count += 1

            m_flat = m_t[:, 0:ch, :, :].rearrange("p x c v -> p (x c v)")
            tail_i = it - (len(CHUNKS) - SYM_ACT_TAIL)
            if tail_i < 0:
                for k in range((ch * Cc * VT) // SYM_MM_COLS):
                    nc.tensor.matmul(
                        sym_psum[0:1, 0:SYM_MM_COLS],
                        ones_col[:, 0:1],
                        m_flat[:, SYM_MM_COLS * k : SYM_MM_COLS * (k + 1)],
                        start=False,
                        stop=False,
                        tile_position=(0, 0),
                        skip_group_check=True,
                    )
            else:
                # tail iterations: defer the m sums; they are emitted after
                # the loop so they queue on the ACT behind ALL exps (the ACT
                # is a strict in-order FIFO) and run in its idle tail while
                # the PE drains its gram backlog.
                deferred_sym.append((tail_i, m_flat))

        for tail_i, m_flat in deferred_sym:
            ms = d_pool.tile([P, CM * Cc * VT], bf16, tag="ms")
            nc.scalar.activation(
                out=ms[:, 0 : m_flat.shape[1]],
                in_=m_flat,
                func=mybir.ActivationFunctionType.Copy,
                accum_out=sym_cols[:, tail_i : tail_i + 1],
            )
        assert mm_count == XS * VT
        # close the accumulation groups (adds zeros, flips stop)
        nc.tensor.matmul(
            a_psum[:, 0 : 4 * Cc], zeros_ws[:, 0:P], zeros_ws[:, 0 : 4 * Cc],
            start=False, stop=True,
        )
        nc.tensor.matmul(
            sym_psum[0:1, 0:SYM_MM_COLS], zeros_ws[:, 0:1],
            zeros_ws[:, 0:SYM_MM_COLS],
            start=False, stop=True,
        )
        nc.vector.tensor_copy(out=a_sb[:], in_=a_psum[:])
        nc.vector.tensor_copy(out=sym_sb[:], in_=sym_psum[:])
        nc.sync.dma_start(out=a_out[:], in_=a_sb[:])
        nc.sync.dma_start(out=sym_out[:], in_=sym_sb[:])
        nc.sync.dma_start(out=symc_out[:], in_=sym_cols[:])

    # The HWDGE pseudo-DMA has a single sync-wait slot, but a recycled load
    # buffer carries both a WAR wait (previous exp read, Activation sem) and
    # a WAW wait (previous fill, DMAHW sem).  All SP-issued HWDGE DMAs share
    # one physical FIFO ring (qSPDynamicHW), so same-ring WAW ordering is
    # guaranteed by hardware per SDMA engine; drop the redundant DMAHW wait.
    for d in lg_dma_ring:
        si = d.ins.sync_info
        if si is None or si.on_wait is None:
            continue
        ws = list(si.on_wait)
        if len(ws) > 1:
            keep = [w for w in ws if not (w.ant_name or "").startswith("DMAHW")]
            if keep and len(keep) < len(ws):
                si.on_wait = keep

    nc.compile()
    return nc


def _finish_loss(A_b, vol_b, sym_total, age, w_young, w_old,
                 vol_means_young, vol_means_old, vol_stds_young, vol_stds_old,
                 prior_adj):
    """Host-side tiny final math (numpy, float64 internally)."""
    alpha = np.clip(age.astype(np.float64) / AGE_MAX, 0.0, 1.0)  # (B,1)

    eye = np.eye(C)
    A = A_b * (1.0 - eye)[None]                                   # zero diag
    W = (1.0 - alpha)[:, :, None] * w_young[None] + alpha[:, :, None] * w_old[None]
    Aw = (A * W).mean(axis=0)
    Aw = Aw / np.clip(Aw.sum(axis=1, keepdims=True), EPS_ROW, None)
    prior = prior_adj * (1.0 - eye)
    prior = prior / np.clip(prior.sum(axis=1, keepdims=True), EPS_ROW, None)
    loss_adj = np.mean(np.abs(Aw - prior))

    means = (1.0 - alpha) * vol_means_young[None] + alpha * vol_means_old[None]
    stds = (1.0 - alpha) * vol_stds_young[None] + alpha * vol_stds_old[None]
    r = (vol_b - means) / (stds + EPS_STD)
    ar = np.abs(r)
    loss_vol = np.mean(np.where(ar < 1.0, 0.5 * r * r, ar - 0.5))

    loss_sym = sym_total / float(B * C * X * Y * Z)

    total = (LAMBDA_WEIGHTED_ADJ * loss_adj
             + LAMBDA_VOLUME * loss_vol
             + LAMBDA_SYM * loss_sym)
    return np.float32(total)


def _shard_for_core(logits, b, q, Cc=C, XS=X, YQc=YQ, Zc=Z):
    """Slice one core's shard and lay it out as [XS, 128, Cc, VT] bf16 with
    voxel v = y*Zc + z mapped to (vt, part) = (v // 128, v % 128)."""
    NV = YQc * Zc
    VT = NV // P
    sh = logits[b, :, :, q * YQc : (q + 1) * YQc, :]      # [C, XS, YQ, Z]
    sh = sh.reshape(Cc, XS, VT, P)                        # v -> (vt, part)
    sh = sh.transpose(1, 3, 0, 2)                         # [XS, part, C, VT]
    import ml_dtypes
    return np.ascontiguousarray(np.asarray(sh, dtype=np.float32).astype(ml_dtypes.float8_e4m3))


_CACHE = {}


def kernel(logits, age, w_young, w_old, vol_means_young, vol_means_old,
           vol_stds_young, vol_stds_old, prior_adj, perm):
    from concourse.bass_utils import run_bass_kernel_spmd

    logits = np.asarray(logits, dtype=np.float32)

    if "nc" not in _CACHE:
        _CACHE["nc"] = build_nc()
    nc = _CACHE["nc"]

    in_maps = []
    for core in range(N_CORES):
        b = core // 4
        q = core % 4
        in_maps.append({"lg": _shard_for_core(logits, b, q)})

    res = run_bass_kernel_spmd(nc, in_maps, core_ids=list(range(N_CORES)))
    _CACHE["last_results"] = res

    A_b = np.zeros((B, C, C), dtype=np.float64)
    sym_total = 0.0
    for core in range(N_CORES):
        b = core // 4
        a_full = res.results[core]["a_out"].astype(np.float64)  # [128, 4*C]
        for j in range(4):
            A_b[b] += a_full[j * C : (j + 1) * C, j * C : (j + 1) * C]
        n_vox_core = X * YQ * Z  # voxels per core (each contributes sum_c p = 1)
        sum_max = float(res.results[core]["sym_out"].astype(np.float64).sum())
        sum_max += float(res.results[core]["symc_out"].astype(np.float64).sum())
        sym_total += 2.0 * (2.0 * sum_max - float(n_vox_core))
    vol_b = A_b.sum(axis=2)  # softmax rows sum to 1 -> row sums give volumes

    return _finish_loss(
        A_b, vol_b, sym_total,
        np.asarray(age), np.asarray(w_young), np.asarray(w_old),
        np.asarray(vol_means_young), np.asarray(vol_means_old),
        np.asarray(vol_stds_young), np.asarray(vol_stds_old),
        np.asarray(prior_adj),
    )
